# revision 13
# baseline (speedup 1.0000x reference)
"""Trainium2 Bass kernel for nn_CrossDomainFusion.

Data-parallel over batch: core b handles batch b (B=8 across 8 cores),
weights replicated.

Math (per batch), exploiting that both value matrices are low-rank:
  T  = conv_transpose(x)                 [2048, 256]   (pre-projection)
  Ht = T @ Wt' + bt                      (bt folded via logit bias + output row)
  Hs = P~^T @ Ws'                        P~ = spec features + ones row [193, 2048]
  S  = Hs @ Ht^T = P~^T @ (M @ T^T) + BL·1^T,  M = Ws' @ Wt  (host)  [193, 256]
  E  = exp(S/sqrt(512))  -> fp8, both orientations (PE transpose)
  fused_time = (E^T @ P~^T-fp8) @ Ws' / dt   (dt = col 192 of R, ones-row trick)
  fused_spec = (E @ [T,1]-fp8) @ Wt' / ds + bt-row

The two big attention-apply matmuls (R = E^T@P~^T, U = E@[T,1]) run as
fp8 DoubleRow matmuls (2 k-tiles of 128 per instruction); contraction
on the projection side is 193/257 instead of 512, roughly halving PE
work vs. the direct H-space formulation. Logit path stays bf16.
"""

import sys

sys.path.insert(0, "/opt/trn_rl_repo")

from contextlib import ExitStack

import ml_dtypes
import numpy as np

import concourse.bacc as bacc
import concourse.tile as tile
from concourse import mybir
from concourse.bass_utils import run_bass_kernel_spmd
from concourse.masks import make_identity

BF16 = mybir.dt.bfloat16
F8 = mybir.dt.float8e4
F32 = mybir.dt.float32
NPBF16 = ml_dtypes.bfloat16
NPF8 = ml_dtypes.float8_e4m3
DR = mybir.MatmulPerfMode.DoubleRow

B, L, C, D, S, CF = 8, 1024, 256, 512, 2048, 192
SCALE = 1.0 / float(np.sqrt(D))
EXP = mybir.ActivationFunctionType.Exp
ADD = mybir.AluOpType.add


def build_nc():
    nc = bacc.Bacc("TRN2", target_bir_lowering=False, debug=False, num_devices=8)
    xt = nc.declare_dram_parameter("xt", [C, L + 2], BF16, isOutput=False)
    wct = nc.declare_dram_parameter("wct", [4, C, C], BF16, isOutput=False)
    mT = nc.declare_dram_parameter("mT", [C, CF + 1], BF16, isOutput=False)
    g0c = nc.declare_dram_parameter("g0c", [CF + 1, 1], BF16, isOutput=False)
    sfa = nc.declare_dram_parameter("sfa", [CF + 1, S], BF16, isOutput=False)
    p8t = nc.declare_dram_parameter("p8t", [8, 128, 2 * (CF + 1)], F8, isOutput=False)
    wsp = nc.declare_dram_parameter("wsp", [CF + 1, D], BF16, isOutput=False)
    wtn = nc.declare_dram_parameter("wtn", [C, D], BF16, isOutput=False)
    btt = nc.declare_dram_parameter("btt", [128, D], F32, isOutput=False)
    out = nc.declare_dram_parameter("out", [S, 2 * D], F32, isOutput=True)
    out_r = out.ap().rearrange("(m two) h -> two m h", two=2)

    with ExitStack() as ctx:
        tc = ctx.enter_context(tile.TileContext(nc))
        const = ctx.enter_context(tc.tile_pool(name="const", bufs=1))
        hpool = ctx.enter_context(tc.tile_pool(name="h", bufs=1))
        epool = ctx.enter_context(tc.tile_pool(name="e", bufs=9))
        tpool = ctx.enter_context(tc.tile_pool(name="t", bufs=3))
        spool = ctx.enter_context(tc.tile_pool(name="stage", bufs=3))
        ps = ctx.enter_context(tc.tile_pool(name="ps", bufs=2, space="PSUM"))
        pe8 = ctx.enter_context(tc.tile_pool(name="pe8", bufs=1, space="PSUM"))
        pr = ctx.enter_context(tc.tile_pool(name="pr", bufs=1, space="PSUM"))
        pu = ctx.enter_context(tc.tile_pool(name="pu", bufs=1, space="PSUM"))
        ptr = ctx.enter_context(tc.tile_pool(name="ptr", bufs=1, space="PSUM"))
        po = ctx.enter_context(tc.tile_pool(name="po", bufs=1, space="PSUM"))

        # ---- input loads ----
        XT = []
        for c in range(2):
            t = const.tile([128, L + 2], BF16, name=f"xt{c}", tag=f"xt{c}")
            nc.sync.dma_start(t[:], xt[c * 128 : (c + 1) * 128, :])
            XT.append(t)
        WCT = []
        for t_ in range(4):
            row = []
            for c in range(2):
                w = const.tile([128, C], BF16, name=f"wct{t_}{c}", tag=f"wct{t_}{c}")
                nc.sync.dma_start(w[:], wct[t_, c * 128 : (c + 1) * 128, :])
                row.append(w)
            WCT.append(row)
        MT = []
        for c in range(2):
            w = const.tile([128, CF + 1], BF16, name=f"mt{c}", tag=f"mt{c}")
            nc.sync.dma_start(w[:], mT[c * 128 : (c + 1) * 128, :])
            MT.append(w)
        G0C0 = const.tile([128, 1], BF16, tag="g0c0")
        nc.sync.dma_start(G0C0[:], g0c[0:128, :])
        G0C1 = const.tile([65, 1], BF16, tag="g0c1")
        nc.sync.dma_start(G0C1[:], g0c[128:193, :])
        SFA0 = const.tile([128, S], BF16, tag="sfa0")
        nc.sync.dma_start(SFA0[:], sfa[0:128, :])
        SFA1 = const.tile([66, S], BF16, tag="sfa1")
        nc.sync.dma_start(SFA1[0:65, :], sfa[128:193, :])
        PT8 = []
        for j in range(8):
            t = const.tile([128, 2, CF + 1], F8, name=f"pt8{j}", tag=f"pt8{j}")
            nc.sync.dma_start(
                t[:], p8t.ap().rearrange("j p (two n) -> j p two n", two=2)[j]
            )
            PT8.append(t)
        WSP0 = const.tile([128, D], BF16, tag="wsp0")
        nc.sync.dma_start(WSP0[:], wsp[0:128, :])
        WSP1 = const.tile([65, D], BF16, tag="wsp1")
        nc.sync.dma_start(WSP1[:], wsp[128:193, :])
        WTN = []
        for c in range(2):
            w = const.tile([128, D], BF16, name=f"wtn{c}", tag=f"wtn{c}")
            nc.sync.dma_start(w[:], wtn[c * 128 : (c + 1) * 128, :])
            WTN.append(w)
        BTT = const.tile([128, D], F32, tag="btt")
        nc.sync.dma_start(BTT[:], btt[:, :])

        IDB = const.tile([128, 128], BF16, tag="idb")
        make_identity(nc, IDB[:])
        ID8 = const.tile([128, 128], F8, tag="id8")
        make_identity(nc, ID8[:])
        warm = const.tile([128, 512], BF16, tag="warm")
        nc.gpsimd.memset(warm[:], 0.0)
        for _ in range(12):
            wp = ps.tile([128, 512], F32, name="wps", tag="ps")
            nc.tensor.matmul(wp[:], lhsT=IDB[:], rhs=warm[:], start=True, stop=True)

        # ---- persistent SBUF tensors ----
        TtT = [hpool.tile([128, S], BF16, name=f"ttt{d}", tag=f"ttt{d}") for d in range(2)]
        G0 = hpool.tile([128, S], BF16, tag="g0")
        G1 = hpool.tile([66, S], BF16, tag="g1")
        TN = [hpool.tile([128, 2, C + 1], F8, name=f"tn{j}", tag=f"tn{j}") for j in range(8)]
        U2 = [hpool.tile([128, C + 1], F32, name=f"u2{k}", tag=f"u2{k}") for k in range(16)]
        RT = hpool.tile([128, 16], F32, tag="rt")
        RS = hpool.tile([128, 16], F32, tag="rs")

        # ---- BL: logit bias row -> SFA1 partition 65 ----
        # BL[s] = sum_cf P~[cf,s] * g0[cf]; lands in SFA row 193 so the S
        # matmul's second k-tile (66 parts) applies it against G1's ones row.
        # DVE cannot shift partitions, so stage at partition 0 and DMA across.
        BLS = hpool.tile([1, S], BF16, tag="bls")
        for sl in range(4):
            pbl = po.tile([1, 512], F32, name="pbl", tag="po")
            nc.tensor.matmul(
                pbl[:], lhsT=G0C0[:], rhs=SFA0[:, sl * 512 : (sl + 1) * 512],
                start=True, stop=False,
            )
            nc.tensor.matmul(
                pbl[:], lhsT=G0C1[:], rhs=SFA1[0:65, sl * 512 : (sl + 1) * 512],
                start=False, stop=True,
            )
            nc.vector.tensor_copy(BLS[:, sl * 512 : (sl + 1) * 512], pbl[:])
        nc.sync.dma_start(SFA1[65:66, :], BLS[:])

        # ---- conv: TtT[d][co, time-layout]; layout = [even 0:1024, odd 1024:2048]
        # wct rows: 0=W1(x[m],even) 1=W3(x[m-1],even) 2=W2(x[m],odd) 3=W0(x[m+1],odd)
        taps = [((0, 1), (1, 0)), ((2, 1), (3, 2))]  # (wct_idx, xt_offset)
        for d in range(2):
            for half in range(2):
                for ms in range(2):
                    p = ps.tile([128, 512], F32, name="cps", tag="ps")
                    n = 0
                    for ti, off in taps[half]:
                        for c in range(2):
                            nc.tensor.matmul(
                                p[:],
                                lhsT=WCT[ti][c][:, d * 128 : (d + 1) * 128],
                                rhs=XT[c][:, off + ms * 512 : off + ms * 512 + 512],
                                start=(n == 0),
                                stop=(n == 3),
                            )
                            n += 1
                    col = half * 1024 + ms * 512
                    if (half + ms) % 2 == 0:
                        nc.scalar.copy(TtT[d][:, col : col + 512], p[:])
                    else:
                        nc.vector.tensor_copy(TtT[d][:, col : col + 512], p[:])

        # ---- G = M @ T^T (bf16), plus ones row for the BL term ----
        # (full-tile memset: row 65 keeps 1.0, rows 0..64 overwritten below)
        nc.gpsimd.memset(G1[:], 1.0)
        for pg in range(2):
            for tsl in range(4):
                rows = 128 if pg == 0 else 65
                p = ps.tile([rows, 512], F32, name="gps", tag="ps")
                for c in range(2):
                    nc.tensor.matmul(
                        p[:],
                        lhsT=MT[c][:, pg * 128 : pg * 128 + rows],
                        rhs=TtT[c][:, tsl * 512 : (tsl + 1) * 512],
                        start=(c == 0),
                        stop=(c == 1),
                    )
                dst = G0 if pg == 0 else G1[0:65, :]
                if (pg + tsl) % 2 == 0:
                    nc.vector.tensor_copy(dst[:, tsl * 512 : (tsl + 1) * 512], p[:])
                else:
                    nc.scalar.copy(dst[:, tsl * 512 : (tsl + 1) * 512], p[:])

        # ---- TN: T natural-layout fp8 pairs [time-part, pair, 257] ----
        for j in range(8):
            nc.gpsimd.memset(TN[j][:, :, C : C + 1], 1.0)
            tp = ptr.tile([128, 2, 256], BF16, name="tnp", tag="ptr")
            for i in range(2):
                for cc in range(2):
                    nc.tensor.transpose(
                        tp[:, i, cc * 128 : (cc + 1) * 128],
                        TtT[cc][:, (2 * j + i) * 128 : (2 * j + i + 1) * 128],
                        IDB[:],
                    )
            nc.vector.tensor_copy(TN[j][:, :, 0:C], tp[:])

        # ---- attention ----
        def u_step(tsl, sc, EPj, i):
            # eTs: fp8 transposes (element step 2) -> compact fp8, then U DR
            etp = pe8.tile([128, 4, 128, 2], F8, name="etp", tag="pe8")
            for q in range(4):
                nc.tensor.transpose(
                    etp[:, q, :, 0], EPj[:, i, q * 128 : (q + 1) * 128], ID8[:]
                )
            eTs = tpool.tile([128, 4, 128], F8, name="ets", tag="ets")
            if sc % 2 == 0:
                nc.vector.tensor_copy(eTs[:], etp[:, :, :, 0])
            else:
                nc.scalar.copy(eTs[:], etp[:, :, :, 0])
            up = pu.tile([128, C + 1], F32, name="up", tag="pu")
            for u in range(2):
                nc.tensor.matmul(
                    up[:],
                    lhsT=eTs[:, 2 * u : 2 * u + 2, :],
                    rhs=TN[tsl * 2 + u][:],
                    start=(u == 0),
                    stop=(u == 1),
                    perf_mode=DR,
                )
            if tsl == 0:
                nc.vector.tensor_copy(U2[sc][:], up[:])
            else:
                nc.vector.tensor_tensor(U2[sc][:], U2[sc][:], up[:], op=ADD)

        def u_final(sc):
            # fused_spec finalize: normalize, project, bias
            nc.vector.reciprocal(RS[:, sc : sc + 1], U2[sc][:, C : C + 1])
            usb = spool.tile([128, C], BF16, name="usb", tag="usb")
            nc.vector.tensor_copy(usb[:], U2[sc][:, 0:C])
            trp = ptr.tile([128, 2, 256], BF16, name="utp", tag="ptr")
            for cc in range(2):
                nc.tensor.transpose(
                    trp[:, cc, 0:128], usb[:, cc * 128 : (cc + 1) * 128], IDB[:]
                )
            ust = spool.tile([128, 2, 128], BF16, name="ust", tag="ust")
            nc.vector.tensor_copy(ust[:], trp[:, :, 0:128])
            os_ = po.tile([128, D], F32, name="osp", tag="po")
            for cc in range(2):
                nc.tensor.matmul(
                    os_[:], lhsT=ust[:, cc, :], rhs=WTN[cc][:],
                    start=(cc == 0), stop=(cc == 1),
                )
            o2 = spool.tile([128, D], F32, name="o2", tag="o")
            nc.scalar.mul(o2[:], os_[:], RS[:, sc : sc + 1])
            nc.vector.tensor_tensor(o2[:], o2[:], BTT[:], op=ADD)
            nc.sync.dma_start(out[sc * 128 : (sc + 1) * 128, D : 2 * D], o2[:])

        def r_step(tsl, EP, q):
            # R: fp8 DR over all spec pairs (one PSUM accumulation group),
            # then fused_time finalize. Interleaved groups within one bank
            # corrupt each other, hence one group at a time.
            k = tsl * 4 + q
            RP = pr.tile([128, CF + 1], F32, name="rp", tag="rp", bufs=2)
            for j in range(8):
                nc.tensor.matmul(
                    RP[:],
                    lhsT=EP[j][:, :, q * 128 : (q + 1) * 128],
                    rhs=PT8[j][:],
                    start=(j == 0),
                    stop=(j == 7),
                    perf_mode=DR,
                )
            nc.vector.reciprocal(RT[:, k : k + 1], RP[:, CF : CF + 1])
            rsb = spool.tile([128, CF + 1], BF16, name="rsb", tag="rsb")
            nc.vector.tensor_copy(rsb[:], RP[:])
            trp = ptr.tile([128, 2, 256], BF16, name="rtp", tag="ptr")
            nc.tensor.transpose(trp[:, 0, 0:128], rsb[:, 0:128], IDB[:])
            nc.tensor.transpose(trp[0:65, 1, 0:128], rsb[:, 128:193], IDB[:])
            rst0 = spool.tile([128, 128], BF16, name="rst0", tag="rst0")
            nc.vector.tensor_copy(rst0[:], trp[:, 0, 0:128])
            rst1 = spool.tile([65, 128], BF16, name="rst1", tag="rst1")
            nc.vector.tensor_copy(rst1[:], trp[0:65, 1, 0:128])
            ot = po.tile([128, D], F32, name="otp", tag="po")
            nc.tensor.matmul(ot[:], lhsT=rst0[:], rhs=WSP0[:], start=True, stop=False)
            nc.tensor.matmul(ot[:], lhsT=rst1[:], rhs=WSP1[:], start=False, stop=True)
            o1 = spool.tile([128, D], F32, name="o1", tag="o")
            nc.scalar.mul(o1[:], ot[:], RT[:, k : k + 1])
            par, m0 = (0, k * 128) if k < 8 else (1, (k - 8) * 128)
            nc.sync.dma_start(out_r[par, m0 : m0 + 128, 0:D], o1[:])

        for tsl in range(4):
            EP = [None] * 8
            prev = None
            for sc in range(17):
                if sc < 16:
                    j, i = sc // 2, sc % 2
                    # S tile [spec 128, time 512] inc. logit bias via row 193
                    p = ps.tile([128, 512], F32, name="sps", tag="ps")
                    nc.tensor.matmul(
                        p[:], lhsT=SFA0[:, sc * 128 : (sc + 1) * 128],
                        rhs=G0[:, tsl * 512 : (tsl + 1) * 512], start=True, stop=False,
                    )
                    nc.tensor.matmul(
                        p[:], lhsT=SFA1[:, sc * 128 : (sc + 1) * 128],
                        rhs=G1[:, tsl * 512 : (tsl + 1) * 512], start=False, stop=True,
                    )
                    if i == 0:
                        EP[j] = epool.tile([128, 2, 512], F8, name="ep", tag="ep")
                    nc.scalar.activation(EP[j][:, i, :], p[:], EXP, scale=SCALE)
                # software pipeline: transposes/U for sc-1 overlap ACT(sc)
                if prev is not None:
                    psc, pi = prev
                    u_step(tsl, psc, EP[psc // 2], pi)
                    if tsl == 3 and psc < 14:
                        u_final(psc)
                prev = (sc, i) if sc < 16 else None

            if tsl < 3:
                for q in range(4):
                    r_step(tsl, EP, q)
            else:
                # tail: interleave last fused_spec finalizes with R groups
                r_step(tsl, EP, 0)
                u_final(14)
                r_step(tsl, EP, 1)
                u_final(15)
                r_step(tsl, EP, 2)
                r_step(tsl, EP, 3)

    nc.compile()
    return nc


def make_in_maps(
    time_features,
    spec_features,
    w_conv,
    b_conv,
    w_tproj,
    b_tproj,
    w_sproj,
    b_sproj,
):
    time_features = np.asarray(time_features, np.float32)
    spec_features = np.asarray(spec_features, np.float32)
    w_conv = np.asarray(w_conv, np.float32)
    b_conv = np.asarray(b_conv, np.float32)
    w_tproj = np.asarray(w_tproj, np.float32)
    b_tproj = np.asarray(b_tproj, np.float32)
    w_sproj = np.asarray(w_sproj, np.float32)
    b_sproj = np.asarray(b_sproj, np.float32)

    # conv taps (pre-projection), order [W1, W3, W2, W0]
    wct = np.stack(
        [w_conv[:, :, 1], w_conv[:, :, 3], w_conv[:, :, 2], w_conv[:, :, 0]]
    ).astype(NPBF16)
    wsp_aug = np.concatenate([w_sproj.T, b_sproj[None, :]], 0)  # [193, 512]
    bt = b_conv @ w_tproj.T + b_tproj  # [512]
    M = wsp_aug @ w_tproj  # [193, 256]
    mT = np.ascontiguousarray(M.T).astype(NPBF16)  # [256, 193]
    g0 = (wsp_aug @ bt).reshape(CF + 1, 1).astype(NPBF16)
    wsp = wsp_aug.astype(NPBF16)
    wtn = np.ascontiguousarray(w_tproj.T).astype(NPBF16)  # [256, 512]
    btt = np.broadcast_to(bt.astype(np.float32), (128, D)).copy()

    in_maps = []
    for b in range(B):
        xt = np.zeros((C, L + 2), NPBF16)
        xt[:, 1 : L + 1] = time_features[b].T.astype(NPBF16)
        P_aug = np.concatenate(
            [spec_features[b].reshape(CF, S), np.ones((1, S), np.float32)], 0
        )
        sfa = P_aug.astype(NPBF16)
        p8t = np.ascontiguousarray(
            P_aug.T.reshape(8, 2, 128, CF + 1).transpose(0, 2, 1, 3).reshape(
                8, 128, 2 * (CF + 1)
            )
        ).astype(NPF8)
        in_maps.append(
            {
                "xt": xt,
                "wct": wct,
                "mT": mT,
                "g0c": g0,
                "sfa": sfa,
                "p8t": p8t,
                "wsp": wsp,
                "wtn": wtn,
                "btt": btt,
            }
        )
    return in_maps


_NC_CACHE = None


def get_nc():
    global _NC_CACHE
    if _NC_CACHE is None:
        _NC_CACHE = build_nc()
    return _NC_CACHE


def kernel(**inputs) -> np.ndarray:
    nc = get_nc()
    in_maps = make_in_maps(**inputs)
    res = run_bass_kernel_spmd(nc, in_maps, list(range(B)))
    return np.stack([res.results[i]["out"] for i in range(B)])


if __name__ == "__main__":
    rng = np.random.default_rng(0)
    ins = {
        "time_features": rng.standard_normal((B, L, C)).astype(np.float32),
        "spec_features": rng.standard_normal((B, 3, 64, S)).astype(np.float32),
        "w_conv": (rng.standard_normal((C, C, 4)) * 0.05).astype(np.float32),
        "b_conv": (rng.standard_normal(C) * 0.05).astype(np.float32),
        "w_tproj": (rng.standard_normal((D, C)) * 0.05).astype(np.float32),
        "b_tproj": (rng.standard_normal(D) * 0.05).astype(np.float32),
        "w_sproj": (rng.standard_normal((D, CF)) * 0.05).astype(np.float32),
        "b_sproj": (rng.standard_normal(D) * 0.05).astype(np.float32),
    }
    out = kernel(**ins)
    print("out", out.shape, out.dtype, float(np.abs(out).max()))
